# revision 1
# baseline (speedup 1.0000x reference)
"""nn_DCAttention Trainium2 kernel: full inputs -> full output, SPMD over 8 NeuronCores.

Sharding:
  Phase A (projections): token-parallel (8 blocks of 512 tokens; conv halo
  comes in with the pre-transposed input, zero-padded at batch edges).
  A2A #1 re-shards Q/K/V/tau/delta to head-pair-parallel (core c: heads 2c,2c+1).
  Phase B: attention per (batch, head), flash-style streaming over key tiles,
  all in transposed (dims, tokens) layout; softmax without max-subtraction
  (scores are bounded by construction: |raw|/8 * sigmoid + sigmoid).
  A2A #2 re-shards attention output back to token-parallel for out_proj.
All matmuls run in float32r (full PE rate at moving-dim >= 256, ~1e-4 accuracy).
"""
import numpy as np
import concourse.bass as bass
import concourse.tile as tile
import concourse.mybir as mybir
from concourse import bacc

f32 = mybir.dt.float32
f32r = mybir.dt.float32r
AF = mybir.ActivationFunctionType
ALU = mybir.AluOpType

D, H, B, L = 1024, 16, 2, 2048
DK = D // H          # 64
NCORES = 8
T = (B * L) // NCORES  # 512 tokens per core
TH = T + 2             # with halo
KT = D // 128          # 8 k-tiles for D contraction
GROUPS = [[0, 1, 2, 3, 4, 5, 6, 7]]

# A2A #1 shard layout, per head-pair shard (rows x 512):
#   rows 0:128    K^T  (128 dims, 512 tok)
#   rows 128:256  Q^T
#   rows 256:384  V natural (512 tok, 128 dims) viewed as flat
#   rows 384:388  tau'(2 heads) then delta(2 heads)
A2A1_ROWS = 388
A2A2_ROWS = 128
GELU_FUNC = AF.Gelu  # sim lacks Gelu; tests may substitute


def build(debug_outputs=(), repeat=1):
    nc = bacc.Bacc(None, target_bir_lowering=False, debug=False)
    nc.num_devices = NCORES

    dp = lambda name, shape, dtype=f32: nc.declare_dram_parameter(name, list(shape), dtype, isOutput=False)
    xT = dp("xT", (D, TH))                    # x^T with halo, zero-padded
    WqT = dp("WqT", (D, D)); Wq_b = dp("Wq_b", (D,))
    WkT = dp("WkT", (D, D)); Wk_b = dp("Wk_b", (D,))
    WvT = dp("WvT", (D, D)); Wv_b = dp("Wv_b", (D,))
    cqT = dp("cqT", (3, D, D)); cq_b = dp("cq_b", (D,))   # convq_w[:,:,k].T stacked
    ckT = dp("ckT", (3, D, D)); ck_b = dp("ck_b", (D,))
    qpT = dp("qpT", (2 * D, D)); qp_b = dp("qp_b", (D,))
    kpT = dp("kpT", (2 * D, D)); kp_b = dp("kp_b", (D,))
    tau1p = dp("tau1p", (2 * D, 4))           # [w0 w1 w2 b]
    del1p = dp("del1p", (2 * D, 4))
    tau2T = dp("tau2T", (2 * D, H)); tau2_b = dp("tau2_b", (H,))
    del2T = dp("del2T", (2 * D, H)); del2_b = dp("del2_b", (H,))
    outT = dp("outT", (D, D)); out_b = dp("out_b", (D,))
    mask_lo = dp("mask_lo", (1,))   # 0.0 when left halo is outside the batch
    mask_hi = dp("mask_hi", (1,))

    yT = nc.declare_dram_parameter("yT", [D, T], f32, isOutput=True)

    dbg = {}
    for name, shape in [
        ("k_inT", (D, TH)), ("k3T", (D, T)), ("kT_", (D, T)),
        ("q_inT", (D, TH)), ("q3T", (D, T)), ("qT_", (D, T)),
        ("V_", (T, D)), ("tau", (H, T)), ("delta", (H, T)),
        ("a2a1_out", (NCORES, A2A1_ROWS, T)), ("attnT", (D, T)),
    ]:
        if name in debug_outputs:
            dbg[name] = nc.declare_dram_parameter("dbg_" + name, list(shape), f32, isOutput=True)

    a2a1_in = nc.dram_tensor("a2a1_in", [NCORES, A2A1_ROWS, T], f32r)
    a2a1_out = nc.dram_tensor("a2a1_out", [NCORES, A2A1_ROWS, T], f32r)
    a2a2_in = nc.dram_tensor("a2a2_in", [NCORES, A2A2_ROWS, T], f32r)
    a2a2_out = nc.dram_tensor("a2a2_out", [NCORES, A2A2_ROWS, T], f32r)

    env = dict(locals())
    with tile.TileContext(nc) as tc:
        for _rep in range(repeat):
            _body(nc, tc, env)
    nc.finalize()
    return nc, dbg


def _body(nc, tc, env):
    g = lambda n: env[n]
    xT, yT, dbg = g("xT"), g("yT"), g("dbg")
    a2a1_in, a2a1_out, a2a2_in, a2a2_out = g("a2a1_in"), g("a2a1_out"), g("a2a2_in"), g("a2a2_out")

    with (
        tc.tile_pool(name="xp", bufs=1) as xp,            # x^T rounded, persistent
        tc.tile_pool(name="const", bufs=1) as constp,
        tc.tile_pool(name="wpool", bufs=2) as wpool,      # streamed weight slices
        tc.tile_pool(name="cwpool", bufs=6) as cwpool,    # conv weights (3 taps live)
        tc.tile_pool(name="vwpool", bufs=2) as vwpool,
        tc.tile_pool(name="actp", bufs=1) as actp,        # k_inT / K3T (reused for q)
        tc.tile_pool(name="evp", bufs=4) as evp,          # psum eviction tiles
        tc.tile_pool(name="tdp", bufs=3) as tdp,          # tau/delta working tiles
        tc.tile_pool(name="ps", bufs=4, space="PSUM") as ps,
        tc.tile_pool(name="ps_td", bufs=2, space="PSUM") as ps_td,
    ):
        # ---- load x^T directly as f32r ----
        xr = xp.tile([128, KT, TH], f32r, tag="xr")
        nc.sync.dma_start(out=xr[:], in_=xT.rearrange("(kt p) t -> p kt t", p=128).bitcast(f32r))

        # ---- biases (per-partition column tiles) ----
        def load_col(name, n=1024):
            t_ = constp.tile([128, n // 128], f32, tag="bias_" + name)
            nc.sync.dma_start(out=t_[:], in_=g(name).rearrange("(mt p) -> p mt", p=128))
            return t_
        b_wq, b_wk = load_col("Wq_b"), load_col("Wk_b")
        b_cq, b_ck = load_col("cq_b"), load_col("ck_b")
        b_qp, b_kp = load_col("qp_b"), load_col("kp_b")
        bv = constp.tile([128, 1024], f32, tag="bv")
        nc.sync.dma_start(out=bv[:], in_=g("Wv_b").ap().unsqueeze(0).broadcast_to([128, 1024]))
        b_tau2 = constp.tile([16, 1], f32, tag="b_tau2")
        nc.sync.dma_start(out=b_tau2[:], in_=g("tau2_b").rearrange("(p o) -> p o", o=1))
        b_del2 = constp.tile([16, 1], f32, tag="b_del2")
        nc.sync.dma_start(out=b_del2[:], in_=g("del2_b").rearrange("(p o) -> p o", o=1))
        m_lo = constp.tile([128, 1], f32, tag="m_lo")
        nc.sync.dma_start(out=m_lo[:], in_=g("mask_lo").ap().unsqueeze(0).broadcast_to([128, 1]))
        m_hi = constp.tile([128, 1], f32, tag="m_hi")
        nc.sync.dma_start(out=m_hi[:], in_=g("mask_hi").ap().unsqueeze(0).broadcast_to([128, 1]))

        def stream_w(pool, ap, cin, mt, mwidth=128, tag="w"):
            """DMA (cin, mwidth) slice for output tile mt -> (128, cin//128, mwidth) f32r."""
            wt = pool.tile([128, cin // 128, mwidth], f32r, tag=tag)
            nc.sync.dma_start(
                out=wt[:],
                in_=ap[:, mt * mwidth:(mt + 1) * mwidth]
                .rearrange("(kt p) m -> p kt m", p=128).bitcast(f32r))
            return wt

        NCH = [(0, 512), (512, 2)]  # halo-width N chunks

        def branch(WT, b_w, cT, b_c, pT, b_p, qk_row0, pref):
            """Q or K branch: linear -> conv3 -> proj; writes proj^T tiles into a2a1_in."""
            in_t = actp.tile([128, KT, TH], f32r, tag="lin")
            for mt in range(KT):
                wt = stream_w(wpool, WT, D, mt, tag="lin_w")
                for (n0, nw) in NCH:
                    p = ps.tile([128, 512], f32, tag="pA")
                    for kt in range(KT):
                        nc.tensor.matmul(p[:, :nw], wt[:, kt, :], xr[:, kt, n0:n0 + nw],
                                         start=(kt == 0), stop=(kt == KT - 1))
                    nc.vector.tensor_scalar_add(in_t[:, mt, n0:n0 + nw], p[:, :nw],
                                                b_w[:, mt:mt + 1])
                # conv zero-padding: kill halo columns outside the batch
                nc.vector.tensor_scalar(in_t[:, mt, 0:1], in_t[:, mt, 0:1],
                                        m_lo[:, 0:1], None, op0=ALU.mult)
                nc.vector.tensor_scalar(in_t[:, mt, TH - 1:TH], in_t[:, mt, TH - 1:TH],
                                        m_hi[:, 0:1], None, op0=ALU.mult)
            if pref + "_inT" in dbg:
                for kt in range(KT):
                    nc.sync.dma_start(out=dbg[pref + "_inT"][kt * 128:(kt + 1) * 128, :],
                                      in_=in_t[:, kt, :].bitcast(f32))
            c3 = actp.tile([128, KT, T], f32r, tag="c3")
            for mt in range(KT):
                wts = [stream_w(cwpool, cT[k], D, mt, tag="c_w") for k in range(3)]
                p = ps.tile([128, 512], f32, tag="pA")
                for kt in range(KT):
                    for k in range(3):
                        nc.tensor.matmul(p[:], wts[k][:, kt, :], in_t[:, kt, k:k + T],
                                         start=(kt == 0 and k == 0), stop=(kt == KT - 1 and k == 2))
                nc.vector.tensor_scalar_add(c3[:, mt, :], p[:], b_c[:, mt:mt + 1])
            if pref + "3T" in dbg:
                for kt in range(KT):
                    nc.sync.dma_start(out=dbg[pref + "3T"][kt * 128:(kt + 1) * 128, :],
                                      in_=c3[:, kt, :].bitcast(f32))
            for mt in range(KT):
                wt = stream_w(wpool, pT, 2 * D, mt, tag="proj_w")
                p = ps.tile([128, 512], f32, tag="pA")
                for kt in range(KT):
                    nc.tensor.matmul(p[:], wt[:, kt, :], in_t[:, kt, 1:1 + T],
                                     start=(kt == 0), stop=False)
                for kt in range(KT):
                    nc.tensor.matmul(p[:], wt[:, KT + kt, :], c3[:, kt, :],
                                     start=False, stop=(kt == KT - 1))
                ev = evp.tile([128, T], f32r, tag="ev")
                nc.vector.tensor_scalar_add(ev[:], p[:], b_p[:, mt:mt + 1])
                nc.sync.dma_start(out=a2a1_in[mt, qk_row0:qk_row0 + 128, :], in_=ev[:])
                if pref + "T_" in dbg:
                    nc.sync.dma_start(out=dbg[pref + "T_"][mt * 128:(mt + 1) * 128, :],
                                      in_=ev[:].bitcast(f32))

        # ---- K branch, V, tau/delta, Q branch ----
        branch(g("WkT"), b_wk, g("ckT"), b_ck, g("kpT"), b_kp, 0, "k")

        # V: natural layout (token, dim)
        for nchunk in range(2):
            wt = stream_w(vwpool, g("WvT"), D, nchunk, mwidth=512, tag="v_w")
            for tt in range(T // 128):
                p = ps.tile([128, 512], f32, tag="pA")
                for kt in range(KT):
                    nc.tensor.matmul(p[:], xr[:, kt, 1 + tt * 128:1 + tt * 128 + 128],
                                     wt[:, kt, :], start=(kt == 0), stop=(kt == KT - 1))
                ev = evp.tile([128, 512], f32r, tag="ev")
                nc.vector.tensor_tensor(ev[:], p[:], bv[:, nchunk * 512:nchunk * 512 + 512], ALU.add)
                for j in range(4):
                    hp = nchunk * 4 + j
                    vsec = a2a1_in[hp, 256:384, :].rearrange("r t -> (r t)").rearrange(
                        "(t d) -> t d", d=128)
                    nc.sync.dma_start(out=vsec[tt * 128:(tt + 1) * 128, :],
                                      in_=ev[:, j * 128:(j + 1) * 128])
                if "V_" in dbg:
                    nc.sync.dma_start(
                        out=dbg["V_"][tt * 128:(tt + 1) * 128, nchunk * 512:(nchunk + 1) * 512],
                        in_=ev[:].bitcast(f32))

        def td_path(p1name, t2name, bias_t, out_row, scale, dbg_name):
            pacc = ps_td.tile([16, 512], f32, tag="ptd")
            p1 = constp.tile([128, 16, 4], f32, tag="p1_" + p1name)
            nc.sync.dma_start(out=p1[:], in_=g(p1name).rearrange("(g p) c -> p g c", p=128))
            for gi in range(16):
                xd = tdp.tile([128, TH], f32, tag="xd")
                nc.sync.dma_start(out=xd[0:64, :], in_=xT[gi * 64:(gi + 1) * 64, :])
                nc.sync.dma_start(out=xd[64:128, :], in_=xT[gi * 64:(gi + 1) * 64, :])
                mid = tdp.tile([128, T], f32, tag="mid")
                nc.vector.tensor_scalar(mid[:], xd[:, 0:T], p1[:, gi, 0:1], None, op0=ALU.mult)
                nc.vector.scalar_tensor_tensor(mid[:], xd[:, 1:1 + T], p1[:, gi, 1:2], mid[:],
                                               op0=ALU.mult, op1=ALU.add)
                nc.vector.scalar_tensor_tensor(mid[:], xd[:, 2:2 + T], p1[:, gi, 2:3], mid[:],
                                               op0=ALU.mult, op1=ALU.add)
                gact = tdp.tile([128, T], f32r, tag="gact")
                nc.scalar.activation(out=gact[:], in_=mid[:], func=GELU_FUNC,
                                     bias=p1[:, gi, 3:4], scale=1.0)
                w2 = wpool.tile([128, 16], f32r, tag="td2")
                nc.sync.dma_start(out=w2[:], in_=g(t2name)[gi * 128:(gi + 1) * 128, :].bitcast(f32r))
                nc.tensor.matmul(pacc[:], w2[:], gact[:], start=(gi == 0), stop=(gi == 15))
            row = tdp.tile([16, T], f32, tag="td_row")
            nc.scalar.activation(out=row[:], in_=pacc[:], func=AF.Sigmoid, bias=bias_t[:, 0:1])
            rowr = tdp.tile([16, T], f32r, tag="td_rowr")
            nc.vector.tensor_scalar(rowr[:], row[:], float(scale), None, op0=ALU.mult)
            nc.sync.dma_start(out=a2a1_in[:, out_row:out_row + 2, :], in_=rowr[:])
            if dbg_name in dbg:
                nc.sync.dma_start(out=dbg[dbg_name][:], in_=rowr[:].bitcast(f32))

        td_path("tau1p", "tau2T", b_tau2, 384, 0.125, "tau")
        td_path("del1p", "del2T", b_del2, 386, 1.0, "delta")

        branch(g("WqT"), b_wq, g("cqT"), b_cq, g("qpT"), b_qp, 128, "q")

    # ---- A2A #1 ----
    nc.gpsimd.collective_compute("AllToAll", ALU.bypass, replica_groups=GROUPS,
                                 ins=[a2a1_in[:]], outs=[a2a1_out[:]])
    if "a2a1_out" in dbg:
        nc.sync.dma_start(out=dbg["a2a1_out"][:], in_=a2a1_out[:].bitcast(f32))

    # ---- Phase B: attention per (batch, head-within-pair) ----
    with (
        tc.tile_pool(name="hconst", bufs=1) as hcp,
        tc.tile_pool(name="hp", bufs=2) as hp_pool,
        tc.tile_pool(name="ep", bufs=4) as ep,
        tc.tile_pool(name="op", bufs=3) as op_pool,
        tc.tile_pool(name="ps_s", bufs=3, space="PSUM") as ps_s,
        tc.tile_pool(name="ps_o", bufs=2, space="PSUM") as ps_o,
        tc.tile_pool(name="ps_b", bufs=1, space="PSUM") as ps_b,
    ):
        ones64f = hcp.tile([1, 64], f32, tag="ones64f")
        nc.vector.memset(ones64f[:], 1.0)
        ones64 = hcp.tile([1, 64], f32r, tag="ones64")
        nc.vector.tensor_copy(out=ones64[:], in_=ones64f[:])
        onescol = hcp.tile([128, 16], f32, tag="onescol")
        nc.vector.memset(onescol[:], 1.0)
        for b in range(2):
            for hh in range(2):
                blk0 = 4 * b
                kts = hp_pool.tile([64, 4, T], f32r, tag="kts")
                nc.sync.dma_start(out=kts[:], in_=a2a1_out[blk0:blk0 + 4, hh * 64:hh * 64 + 64, :]
                                  .transpose([1, 0, 2]))
                qts = hp_pool.tile([64, 4, T], f32r, tag="qts")
                nc.sync.dma_start(out=qts[:],
                                  in_=a2a1_out[blk0:blk0 + 4, 128 + hh * 64:128 + hh * 64 + 64, :]
                                  .transpose([1, 0, 2]))
                vt = hp_pool.tile([128, 16, 65], f32r, tag="vt")
                nc.vector.tensor_copy(out=vt[:, :, 64:65], in_=onescol.unsqueeze(2))
                for j in range(4):
                    vsec = a2a1_out[blk0 + j, 256:384, :].rearrange("r t -> (r t)").rearrange(
                        "(a p d) -> p a d", p=128, d=128)
                    nc.sync.dma_start(out=vt[:, j * 4:(j + 1) * 4, 0:64],
                                      in_=vsec[:, :, hh * 64:hh * 64 + 64])
                taur = hp_pool.tile([1, 4, T], f32r, tag="taur")
                nc.sync.dma_start(out=taur[:],
                                  in_=a2a1_out[blk0:blk0 + 4, 384 + hh:384 + hh + 1, :]
                                  .transpose([1, 0, 2]))
                delt = hp_pool.tile([128, 4, 4], f32, tag="delt")
                for j in range(4):
                    nc.sync.dma_start(
                        out=delt[:, j, :],
                        in_=a2a1_out[blk0 + j, 386 + hh, :].bitcast(f32)
                        .rearrange("(a p) -> p a", p=128))
                qs = hp_pool.tile([64, 4, T], f32r, tag="qs")
                for qc in range(4):
                    pb = ps_b.tile([64, T], f32, tag="pb")
                    nc.tensor.matmul(pb[:], ones64[:], taur[:, qc, :], start=True, stop=True)
                    nc.vector.tensor_tensor(qs[:, qc, :], qts[:, qc, :], pb[:], ALU.mult)
                kflat = kts.rearrange("p a t -> p (a t)")
                for qc in range(4):
                    po = ps_o.tile([65, T], f32, tag="po")
                    for kt in range(16):
                        s = ps_s.tile([128, T], f32, tag="s")
                        nc.tensor.matmul(s[:], kflat[:, kt * 128:(kt + 1) * 128],
                                         qs[:, qc, :], start=True, stop=True)
                        e = ep.tile([128, T], f32r, tag="e")
                        nc.scalar.activation(out=e[:], in_=s[:], func=AF.Exp,
                                             bias=delt[:, kt // 4, kt % 4:kt % 4 + 1], scale=1.0)
                        nc.tensor.matmul(po[:], vt[:, kt, :], e[:],
                                         start=(kt == 0), stop=(kt == 15))
                    rs = op_pool.tile([1, T], f32r, tag="rs")
                    with nc.allow_low_precision(reason="f32r reciprocal for softmax denom"):
                        nc.vector.reciprocal(out=rs[:], in_=po[64:65, :])
                    pb2 = ps_b.tile([64, T], f32, tag="pb2")
                    nc.tensor.matmul(pb2[:], ones64[:], rs[:], start=True, stop=True)
                    rb = op_pool.tile([64, T], f32, tag="rb")
                    nc.vector.tensor_copy(out=rb[:], in_=pb2[:])
                    ot = op_pool.tile([64, T], f32r, tag="ot")
                    nc.vector.tensor_tensor(ot[:], po[0:64, :], rb[:], ALU.mult)
                    nc.sync.dma_start(out=a2a2_in[b * 4 + qc, hh * 64:hh * 64 + 64, :], in_=ot[:])

    # ---- A2A #2 ----
    nc.gpsimd.collective_compute("AllToAll", ALU.bypass, replica_groups=GROUPS,
                                 ins=[a2a2_in[:]], outs=[a2a2_out[:]])
    if "attnT" in dbg:
        nc.sync.dma_start(out=dbg["attnT"][:],
                          in_=a2a2_out.rearrange("s r t -> (s r) t").bitcast(f32))

    # ---- Phase C: out_proj ----
    with (
        tc.tile_pool(name="cw", bufs=3) as cw,
        tc.tile_pool(name="cin", bufs=1) as cin,
        tc.tile_pool(name="cev", bufs=3) as cev,
        tc.tile_pool(name="ps_c", bufs=4, space="PSUM") as ps_c,
    ):
        at = cin.tile([128, KT, T], f32r, tag="at")
        nc.sync.dma_start(out=at[:], in_=a2a2_out.rearrange("s (q p) t -> p (s q) t", p=128))
        bias_out = cin.tile([128, KT], f32, tag="bias_out2")
        nc.sync.dma_start(out=bias_out[:], in_=g("out_b").rearrange("(mt p) -> p mt", p=128))
        for mt in range(KT):
            wt = cw.tile([128, KT, 128], f32r, tag="ow")
            nc.sync.dma_start(out=wt[:], in_=g("outT")[:, mt * 128:(mt + 1) * 128]
                              .rearrange("(kt p) m -> p kt m", p=128).bitcast(f32r))
            p = ps_c.tile([128, T], f32, tag="pc")
            for kt in range(KT):
                nc.tensor.matmul(p[:], wt[:, kt, :], at[:, kt, :],
                                 start=(kt == 0), stop=(kt == KT - 1))
            ev = cev.tile([128, T], f32, tag="cev")
            nc.vector.tensor_scalar_add(ev[:], p[:], bias_out[:, mt:mt + 1])
            nc.sync.dma_start(out=yT[mt * 128:(mt + 1) * 128, :], in_=ev[:])


def make_inputs(full):
    """full: dict of original reference inputs -> list of 8 per-core in_maps."""
    x = np.asarray(full["x"], dtype=np.float32)
    common = {
        "WqT": full["Wq_w"].T, "Wq_b": full["Wq_b"],
        "WkT": full["Wk_w"].T, "Wk_b": full["Wk_b"],
        "WvT": full["Wv_w"].T, "Wv_b": full["Wv_b"],
        "cqT": np.asarray(full["convq_w"]).transpose(2, 1, 0), "cq_b": full["convq_b"],
        "ckT": np.asarray(full["convk_w"]).transpose(2, 1, 0), "ck_b": full["convk_b"],
        "qpT": full["qproj_w"].T, "qp_b": full["qproj_b"],
        "kpT": full["kproj_w"].T, "kp_b": full["kproj_b"],
        "tau1p": np.concatenate([np.asarray(full["tau1_w"])[:, 0, :],
                                 np.asarray(full["tau1_b"])[:, None]], axis=1),
        "del1p": np.concatenate([np.asarray(full["del1_w"])[:, 0, :],
                                 np.asarray(full["del1_b"])[:, None]], axis=1),
        "tau2T": np.asarray(full["tau2_w"])[:, :, 0].T, "tau2_b": full["tau2_b"],
        "del2T": np.asarray(full["del2_w"])[:, :, 0].T, "del2_b": full["del2_b"],
        "outT": full["out_w"].T, "out_b": full["out_b"],
    }
    perm = np.concatenate([g * 128 + np.concatenate([np.arange(0, 128, 2), np.arange(1, 128, 2)])
                           for g in range(16)])
    for k in ["tau1p", "del1p", "tau2T", "del2T"]:
        common[k] = np.asarray(common[k])[perm]
    common = {k: np.ascontiguousarray(np.asarray(v, dtype=np.float32)) for k, v in common.items()}
    ins = []
    for c in range(NCORES):
        b, t0 = c // 4, (c % 4) * T
        xb = np.zeros((TH, D), np.float32)
        lo, hi = max(t0 - 1, 0), min(t0 + T + 1, L)
        xb[lo - (t0 - 1):hi - (t0 - 1)] = x[b, lo:hi]
        m = dict(common)
        m["xT"] = np.ascontiguousarray(xb.T)
        m["mask_lo"] = np.array([0.0 if t0 == 0 else 1.0], np.float32)
        m["mask_hi"] = np.array([0.0 if t0 + T == L else 1.0], np.float32)
        ins.append(m)
    return ins


def assemble(results):
    y = np.empty((B, L, D), np.float32)
    for c in range(NCORES):
        b, t0 = c // 4, (c % 4) * T
        y[b, t0:t0 + T] = results[c]["yT"].T
    return y


def kernel(**inputs):
    """Takes the full unsharded reference inputs, returns the full (B, L, D) output."""
    from concourse.bass_utils import run_bass_kernel_spmd
    nc, _ = build()
    in_maps = make_inputs(inputs)
    res = run_bass_kernel_spmd(nc, in_maps, list(range(NCORES)))
    return assemble(res.results)



# revision 2
# speedup vs baseline: 2.4069x; 2.4069x over previous
"""nn_DCAttention v2: fused conv taps, bf16 collectives, overlapped A2As.

Per-core plan (core c: batch b=c//4, tokens t0=(c%4)*512):
  Phase A (token-parallel):
    tau/delta depthwise path (DVE+gpsimd mids, Act gelu) emitted first;
    K branch = 3-tap conv with host-precomputed taps Ek = kp2@Ck@Wk (+kp1@Wk
    at center tap); V; tau/delta matmuls; -> a2a1a (K,V,tau,delta, bf16);
    Q branch (overlaps a2a1a) -> a2a1b (Q, bf16).
  Phase B (head-pair-parallel): per (b,hh): scores with delta folded in as a
    65th contraction row, tau folded into qs by DMA-broadcast; exp over
    3-bank PSUM groups (N=1536); flash accumulate with ones-column denom.
    b=0 results -> a2a2a (overlapped under b=1 compute).
  Phase C (token-parallel, 256+256 tokens): out_proj bf16, two half passes.
"""
import numpy as np
import ml_dtypes
import concourse.bass as bass
import concourse.tile as tile
import concourse.mybir as mybir
from concourse import bacc

f32 = mybir.dt.float32
f32r = mybir.dt.float32r
bf16 = mybir.dt.bfloat16
AF = mybir.ActivationFunctionType
ALU = mybir.AluOpType

D, H, B, L = 1024, 16, 2, 2048
DK = D // H          # 64
NCORES = 8
T = (B * L) // NCORES  # 512 tokens per core
TH = T + 2             # with halo
KT = D // 128          # 8 k-tiles for D contraction
GROUPS = [[0, 1, 2, 3, 4, 5, 6, 7]]

# a2a1a rows: 0:128 K^T, 128:256 V flat (tok,dim), 256:258 tau' (2 heads),
# 258:260 delta. a2a1b rows: Q^T.
A1A_ROWS = 260
A1B_ROWS = 128
GELU_FUNC = AF.Gelu
KT_GROUPS = [(0, 3), (3, 3), (6, 3), (9, 3), (12, 3), (15, 1)]


def build(debug_outputs=(), repeat=1):
    nc = bacc.Bacc(None, target_bir_lowering=False, debug=False)
    nc.num_devices = NCORES

    dp = lambda name, shape, dtype=f32: nc.declare_dram_parameter(name, list(shape), dtype, isOutput=False)
    xT = dp("xT", (D, TH))                    # x^T with halo, zero-padded
    EqT = dp("EqT", (3, D, D)); bq = dp("bq", (D,)); eq_edge = dp("eq_edge", (D, 2))
    EkT = dp("EkT", (3, D, D)); bk = dp("bk", (D,)); ek_edge = dp("ek_edge", (D, 2))
    WvT = dp("WvT", (D, D)); Wv_b = dp("Wv_b", (D,))
    tau1p = dp("tau1p", (2 * D, 4))           # [w0 w1 w2 b], perm'd
    del1p = dp("del1p", (2 * D, 4))
    tau2T = dp("tau2T", (2 * D, H), bf16); tau2_b = dp("tau2_b", (H,))
    del2T = dp("del2T", (2 * D, H), bf16); del2_b = dp("del2_b", (H,))
    outTb = dp("outTb", (D, D), bf16); out_b = dp("out_b", (D,))

    yT = nc.declare_dram_parameter("yT", [D, T], f32, isOutput=True)

    dbg = {}
    for name, shape, dt_ in [
        ("kT_", (D, T), bf16), ("qT_", (D, T), bf16), ("V_", (T, D), bf16),
        ("tau", (H, T), bf16), ("delta", (H, T), bf16),
        ("a1a_out", (NCORES, A1A_ROWS, T), bf16),
        ("a1b_out", (NCORES, A1B_ROWS, T), bf16),
        ("a2a_out", (NCORES, 128, 256), bf16),
        ("a2b_out", (NCORES, 128, 256), bf16),
    ]:
        if name in debug_outputs:
            dbg[name] = nc.declare_dram_parameter("dbg_" + name, list(shape), dt_, isOutput=True)

    a1a_in = nc.dram_tensor("a1a_in", [NCORES, A1A_ROWS, T], bf16)
    a1a_out = nc.dram_tensor("a1a_out", [NCORES, A1A_ROWS, T], bf16)
    a1b_in = nc.dram_tensor("a1b_in", [NCORES, A1B_ROWS, T], bf16)
    a1b_out = nc.dram_tensor("a1b_out", [NCORES, A1B_ROWS, T], bf16)
    a2a_in = nc.dram_tensor("a2a_in", [NCORES, 128, 256], bf16)
    a2a_out = nc.dram_tensor("a2a_out", [NCORES, 128, 256], bf16)
    a2b_in = nc.dram_tensor("a2b_in", [NCORES, 128, 256], bf16)
    a2b_out = nc.dram_tensor("a2b_out", [NCORES, 128, 256], bf16)

    env = dict(locals())
    with tile.TileContext(nc) as tc:
        for _rep in range(repeat):
            _body(nc, tc, env)
    nc.finalize()
    return nc, dbg


def _body(nc, tc, env):
    g = lambda n: env[n]
    xT, yT, dbg = g("xT"), g("yT"), g("dbg")
    a1a_in, a1a_out = g("a1a_in"), g("a1a_out")
    a1b_in, a1b_out = g("a1b_in"), g("a1b_out")
    a2a_in, a2a_out = g("a2a_in"), g("a2a_out")
    a2b_in, a2b_out = g("a2b_in"), g("a2b_out")

    # ============================ Phase A ============================
    with (
        tc.tile_pool(name="xp", bufs=1) as xp,
        tc.tile_pool(name="const", bufs=1) as constp,
        tc.tile_pool(name="cwpool", bufs=6) as cwpool,    # conv taps (3 live)
        tc.tile_pool(name="vwpool", bufs=2) as vwpool,
        tc.tile_pool(name="evp", bufs=4) as evp,          # psum eviction tiles
        tc.tile_pool(name="tdx", bufs=4) as tdx,          # xd tiles (rotating)
        tc.tile_pool(name="tmid", bufs=2) as tmid,        # mids (rotating per tag)
        tc.tile_pool(name="tga", bufs=1) as tga,          # gacts (all live, bf16)
        tc.tile_pool(name="tde", bufs=2) as tde,
        tc.tile_pool(name="ps", bufs=4, space="PSUM") as ps,
        tc.tile_pool(name="ps_td", bufs=2, space="PSUM") as ps_td,
    ):
        # ---- x^T as f32r ----
        xr = xp.tile([128, KT, TH], f32r, tag="xr")
        nc.sync.dma_start(out=xr[:], in_=xT.rearrange("(kt p) t -> p kt t", p=128).bitcast(f32r))

        def load_col(name, n=1024):
            t_ = constp.tile([128, n // 128], f32, tag="bias_" + name)
            nc.sync.dma_start(out=t_[:], in_=g(name).rearrange("(mt p) -> p mt", p=128))
            return t_
        b_q, b_k = load_col("bq"), load_col("bk")
        e_q = constp.tile([128, KT, 2], f32, tag="e_q")
        nc.sync.dma_start(out=e_q[:], in_=g("eq_edge").rearrange("(mt p) c -> p mt c", p=128))
        e_k = constp.tile([128, KT, 2], f32, tag="e_k")
        nc.sync.dma_start(out=e_k[:], in_=g("ek_edge").rearrange("(mt p) c -> p mt c", p=128))
        bv = constp.tile([128, 1024], f32, tag="bv")
        nc.sync.dma_start(out=bv[:], in_=g("Wv_b").ap().unsqueeze(0).broadcast_to([128, 1024]))
        b_tau2 = constp.tile([16, 1], f32, tag="b_tau2")
        nc.sync.dma_start(out=b_tau2[:], in_=g("tau2_b").rearrange("(p o) -> p o", o=1))
        b_del2 = constp.tile([16, 1], f32, tag="b_del2")
        nc.sync.dma_start(out=b_del2[:], in_=g("del2_b").rearrange("(p o) -> p o", o=1))

        # ---- tau/delta elementwise front (emitted first: DVE+Pool+Act) ----
        p1t = constp.tile([128, 16, 4], f32, tag="p1_tau")
        nc.sync.dma_start(out=p1t[:], in_=g("tau1p").rearrange("(g p) c -> p g c", p=128))
        p1d = constp.tile([128, 16, 4], f32, tag="p1_del")
        nc.sync.dma_start(out=p1d[:], in_=g("del1p").rearrange("(g p) c -> p g c", p=128))
        gacts = {}
        for gi in range(16):
            kt_, p0 = gi // 2, (gi % 2) * 64
            xd = tdx.tile([128, TH], f32, tag="xd")
            nc.sync.dma_start(out=xd[0:64, :], in_=xr[p0:p0 + 64, kt_, :].bitcast(f32))
            nc.sync.dma_start(out=xd[64:128, :], in_=xr[p0:p0 + 64, kt_, :].bitcast(f32))
            for pname, p1, eng in (("t", p1t, nc.vector), ("d", p1d, nc.vector)):
                mid = tmid.tile([128, T], f32, tag="mid" + pname)
                eng.tensor_scalar(mid[:], xd[:, 0:T], p1[:, gi, 0:1], None, op0=ALU.mult)
                eng.scalar_tensor_tensor(mid[:], xd[:, 1:1 + T], p1[:, gi, 1:2], mid[:],
                                         op0=ALU.mult, op1=ALU.add)
                eng.scalar_tensor_tensor(mid[:], xd[:, 2:2 + T], p1[:, gi, 2:3], mid[:],
                                         op0=ALU.mult, op1=ALU.add)
                gact = tga.tile([128, T], bf16, tag=f"gact{pname}{gi}")
                nc.scalar.activation(out=gact[:], in_=mid[:], func=GELU_FUNC,
                                     bias=p1[:, gi, 3:4], scale=1.0)
                gacts[(pname, gi)] = gact

        def stream_w(pool, ap, cin, mt, mwidth=128, tag="w"):
            wt = pool.tile([128, cin // 128, mwidth], f32r, tag=tag)
            nc.sync.dma_start(
                out=wt[:],
                in_=ap[:, mt * mwidth:(mt + 1) * mwidth]
                .rearrange("(kt p) m -> p kt m", p=128).bitcast(f32r))
            return wt

        def branch(ET, b_w, edge_t, out_buf, dbg_name):
            """Fused 3-tap conv branch -> out_buf[mt, 0:128, :] (bf16)."""
            for mt in range(KT):
                wts = [stream_w(cwpool, ET[k], D, mt, tag="c_w") for k in range(3)]
                p = ps.tile([128, T], f32, tag="pA")
                for kt in range(KT):
                    for k in range(3):
                        nc.tensor.matmul(p[:], wts[k][:, kt, :], xr[:, kt, k:k + T],
                                         start=(kt == 0 and k == 0),
                                         stop=(kt == KT - 1 and k == 2))
                ev = evp.tile([128, T], bf16, tag="ev")
                nc.vector.tensor_scalar_add(ev[:], p[:], b_w[:, mt:mt + 1])
                nc.vector.tensor_tensor(ev[:, 0:1], ev[:, 0:1], edge_t[:, mt, 0:1], ALU.add)
                nc.vector.tensor_tensor(ev[:, T - 1:T], ev[:, T - 1:T], edge_t[:, mt, 1:2], ALU.add)
                nc.sync.dma_start(out=out_buf[mt, 0:128, :], in_=ev[:])
                if dbg_name in dbg:
                    nc.sync.dma_start(out=dbg[dbg_name][mt * 128:(mt + 1) * 128, :], in_=ev[:])

        # ---- K branch ----
        branch(g("EkT"), b_k, e_k, a1a_in, "kT_")

        # ---- tau/delta matmuls + evict ----
        for pname, t2name, bias_t, out_row, scale, dbg_name in (
                ("t", "tau2T", b_tau2, 256, 0.125, "tau"),
                ("d", "del2T", b_del2, 258, 1.0, "delta")):
            pacc = ps_td.tile([16, T], f32, tag="ptd")
            for gi in range(16):
                w2 = tde.tile([128, 16], bf16, tag="td2")
                nc.sync.dma_start(out=w2[:], in_=g(t2name)[gi * 128:(gi + 1) * 128, :])
                nc.tensor.matmul(pacc[:], w2[:], gacts[(pname, gi)][:],
                                 start=(gi == 0), stop=(gi == 15))
            row = tde.tile([16, T], f32, tag="td_row")
            nc.scalar.activation(out=row[:], in_=pacc[:], func=AF.Sigmoid, bias=bias_t[:, 0:1])
            rowr = tde.tile([16, T], bf16, tag="td_rowr")
            nc.vector.tensor_scalar(rowr[:], row[:], float(scale), None, op0=ALU.mult)
            nc.sync.dma_start(out=a1a_in[:, out_row:out_row + 2, :], in_=rowr[:])
            if dbg_name in dbg:
                nc.sync.dma_start(out=dbg[dbg_name][:], in_=rowr[:])

        # ---- V (natural layout) ----
        for nchunk in range(2):
            wt = stream_w(vwpool, g("WvT"), D, nchunk, mwidth=512, tag="v_w")
            for tt in range(T // 128):
                p = ps.tile([128, 512], f32, tag="pA")
                for kt in range(KT):
                    nc.tensor.matmul(p[:], xr[:, kt, 1 + tt * 128:1 + tt * 128 + 128],
                                     wt[:, kt, :], start=(kt == 0), stop=(kt == KT - 1))
                ev = evp.tile([128, 512], bf16, tag="ev")
                nc.vector.tensor_tensor(ev[:], p[:], bv[:, nchunk * 512:nchunk * 512 + 512], ALU.add)
                for j in range(4):
                    hp = nchunk * 4 + j
                    vsec = a1a_in[hp, 128:256, :].rearrange("r t -> (r t)").rearrange(
                        "(t d) -> t d", d=128)
                    nc.sync.dma_start(out=vsec[tt * 128:(tt + 1) * 128, :],
                                      in_=ev[:, j * 128:(j + 1) * 128])
                if "V_" in dbg:
                    nc.sync.dma_start(
                        out=dbg["V_"][tt * 128:(tt + 1) * 128, nchunk * 512:(nchunk + 1) * 512],
                        in_=ev[:])

        # ---- A2A 1a: K, V, tau, delta ----
        nc.gpsimd.collective_compute("AllToAll", ALU.bypass, replica_groups=GROUPS,
                                     ins=[a1a_in[:]], outs=[a1a_out[:]])

        # ---- Q branch ----
        branch(g("EqT"), b_q, e_q, a1b_in, "qT_")

    # ---- A2A 1b: Q ----
    nc.gpsimd.collective_compute("AllToAll", ALU.bypass, replica_groups=GROUPS,
                                 ins=[a1b_in[:]], outs=[a1b_out[:]])
    if "a1a_out" in dbg:
        nc.sync.dma_start(out=dbg["a1a_out"][:], in_=a1a_out[:])
    if "a1b_out" in dbg:
        nc.sync.dma_start(out=dbg["a1b_out"][:], in_=a1b_out[:])

    # ======================= Phase B + C =======================
    with (
        tc.tile_pool(name="hconst", bufs=1) as hcp,
        tc.tile_pool(name="hp", bufs=2) as hp_pool,
        tc.tile_pool(name="ep", bufs=3) as ep,
        tc.tile_pool(name="op", bufs=3) as op_pool,
        tc.tile_pool(name="cw", bufs=1) as cw,
        tc.tile_pool(name="cin", bufs=2) as cin,
        tc.tile_pool(name="cev", bufs=3) as cev,
    ):
        # prefetch out_proj weights + bias during phase B
        wt_all = cw.tile([128, KT, D], bf16, tag="ow")
        nc.sync.dma_start(out=wt_all[:], in_=g("outTb").rearrange("(kt p) m -> p kt m", p=128))
        bias_out = cw.tile([128, KT], f32, tag="bias_out")
        nc.sync.dma_start(out=bias_out[:], in_=g("out_b").rearrange("(mt p) -> p mt", p=128))

        ones64f = hcp.tile([1, 64], f32, tag="ones64f")
        nc.vector.memset(ones64f[:], 1.0)
        ones64 = hcp.tile([1, 64], f32r, tag="ones64")
        nc.vector.tensor_copy(out=ones64[:], in_=ones64f[:])
        onescol = hcp.tile([128, 16], bf16, tag="onescol")
        nc.vector.memset(onescol[:], 1.0)

        with (
            tc.tile_pool(name="ps_s", bufs=2, space="PSUM") as ps_s,
            tc.tile_pool(name="ps_o", bufs=1, space="PSUM") as ps_o,
            tc.tile_pool(name="ps_b", bufs=1, space="PSUM") as ps_b,
        ):
            for b in range(2):
                for hh in range(2):
                    blk0 = 4 * b
                    # kaug: rows 0:64 K dims, row 64 delta
                    kaug = hp_pool.tile([65, 16, 128], bf16, tag="kaug")
                    for s in range(4):
                        nc.sync.dma_start(
                            out=kaug[0:64, s * 4:(s + 1) * 4, :],
                            in_=a1a_out[blk0 + s, hh * 64:hh * 64 + 64, :]
                            .rearrange("d (j k) -> d j k", k=128))
                        nc.sync.dma_start(
                            out=kaug[64:65, s * 4:(s + 1) * 4, :],
                            in_=a1a_out[blk0 + s, 258 + hh:259 + hh, :]
                            .rearrange("o (j k) -> o j k", k=128))
                    qts = hp_pool.tile([64, 4, T], bf16, tag="qts")
                    nc.sync.dma_start(out=qts[:],
                                      in_=a1b_out[blk0:blk0 + 4, hh * 64:hh * 64 + 64, :]
                                      .transpose([1, 0, 2]))
                    taub = hp_pool.tile([64, 4, T], bf16, tag="taub")
                    nc.sync.dma_start(out=taub[:],
                                      in_=a1a_out[blk0:blk0 + 4, 256 + hh, :]
                                      .unsqueeze(0).broadcast_to([64, 4, T]))
                    vt = hp_pool.tile([128, 16, 65], bf16, tag="vt")
                    nc.vector.tensor_copy(out=vt[:, :, 64:65], in_=onescol.unsqueeze(2))
                    for j in range(4):
                        vsec = a1a_out[blk0 + j, 128:256, :].rearrange("r t -> (r t)").rearrange(
                            "(a p d) -> p a d", p=128, d=128)
                        nc.sync.dma_start(out=vt[:, j * 4:(j + 1) * 4, 0:64],
                                          in_=vsec[:, :, hh * 64:hh * 64 + 64])
                    # qs_aug: rows 0:64 q*tau, row 64 ones
                    qs = hp_pool.tile([65, 4, T], bf16, tag="qs")
                    nc.vector.tensor_tensor(qs[0:64, :, :], qts[:], taub[:], ALU.mult)
                    nc.vector.memset(qs[64:65, :, :], 1.0)

                    for qc in range(4):
                        po = ps_o.tile([65, T], f32, tag="po")
                        for (k0, glen) in KT_GROUPS:
                            s_ = ps_s.tile([128, 3, T], f32, tag="s")
                            for j in range(glen):
                                nc.tensor.matmul(s_[:, j, :], kaug[:, k0 + j, :],
                                                 qs[:, qc, :], start=True, stop=True)
                            e = ep.tile([128, 3, T], bf16, tag="e")
                            nc.scalar.activation(out=e[:, 0:glen, :], in_=s_[:, 0:glen, :],
                                                 func=AF.Exp)
                            for j in range(glen):
                                kt = k0 + j
                                nc.tensor.matmul(po[:], vt[:, kt, :], e[:, j, :],
                                                 start=(kt == 0), stop=(kt == 15))
                        rs = op_pool.tile([1, T], f32r, tag="rs")
                        with nc.allow_low_precision(reason="f32r reciprocal for softmax denom"):
                            nc.vector.reciprocal(out=rs[:], in_=po[64:65, :])
                        pb2 = ps_b.tile([64, T], f32, tag="pb2")
                        nc.tensor.matmul(pb2[:], ones64[:], rs[:], start=True, stop=True)
                        rb = op_pool.tile([64, T], f32, tag="rb")
                        nc.vector.tensor_copy(out=rb[:], in_=pb2[:])
                        ot = op_pool.tile([64, T], bf16, tag="ot")
                        nc.vector.tensor_tensor(ot[:], po[0:64, :], rb[:], ALU.mult)
                        dst = a2a_in if b == 0 else a2b_in
                        nc.sync.dma_start(out=dst[2 * qc, hh * 64:hh * 64 + 64, :],
                                          in_=ot[:, 0:256])
                        nc.sync.dma_start(out=dst[2 * qc + 1, hh * 64:hh * 64 + 64, :],
                                          in_=ot[:, 256:512])
                if b == 0:
                    nc.gpsimd.collective_compute("AllToAll", ALU.bypass,
                                                 replica_groups=GROUPS,
                                                 ins=[a2a_in[:]], outs=[a2a_out[:]])
            nc.gpsimd.collective_compute("AllToAll", ALU.bypass, replica_groups=GROUPS,
                                         ins=[a2b_in[:]], outs=[a2b_out[:]])
        if "a2a_out" in dbg:
            nc.sync.dma_start(out=dbg["a2a_out"][:], in_=a2a_out[:])
        if "a2b_out" in dbg:
            nc.sync.dma_start(out=dbg["a2b_out"][:], in_=a2b_out[:])

        # ---- Phase C: out_proj, two half-token passes ----
        with tc.tile_pool(name="ps_c", bufs=3, space="PSUM") as ps_c:
            for half, src in ((0, a2a_out), (1, a2b_out)):
                at = cin.tile([128, KT, 256], bf16, tag="at")
                nc.sync.dma_start(out=at[:], in_=src.rearrange("s (q p) t -> p (s q) t", p=128))
                for mt in range(KT):
                    p = ps_c.tile([128, 256], f32, tag="pc")
                    for kt in range(KT):
                        nc.tensor.matmul(p[:], wt_all[:, kt, mt * 128:(mt + 1) * 128],
                                         at[:, kt, :], start=(kt == 0), stop=(kt == KT - 1))
                    ev = cev.tile([128, 256], f32, tag="cev")
                    nc.vector.tensor_scalar_add(ev[:], p[:], bias_out[:, mt:mt + 1])
                    nc.sync.dma_start(out=yT[mt * 128:(mt + 1) * 128,
                                             half * 256:(half + 1) * 256], in_=ev[:])


def make_inputs(full):
    """full: dict of original reference inputs -> list of 8 per-core in_maps."""
    f = {k: np.asarray(v, dtype=np.float32) for k, v in full.items()}
    x = f["x"]

    def fuse(W, Wb, C, Cb, P, Pb):
        p1, p2 = P[:, :D], P[:, D:]
        E = [p2 @ C[:, :, k] @ W for k in range(3)]
        E[1] = E[1] + p1 @ W
        bias = p1 @ Wb + p2 @ ((C[:, :, 0] + C[:, :, 1] + C[:, :, 2]) @ Wb + Cb) + Pb
        e_lo = -(p2 @ (C[:, :, 0] @ Wb))
        e_hi = -(p2 @ (C[:, :, 2] @ Wb))
        ET = np.stack([E[k].T for k in range(3)])
        return ET, bias, e_lo, e_hi

    EqT, bq, eq_lo, eq_hi = fuse(f["Wq_w"], f["Wq_b"], f["convq_w"], f["convq_b"],
                                 f["qproj_w"], f["qproj_b"])
    EkT, bk, ek_lo, ek_hi = fuse(f["Wk_w"], f["Wk_b"], f["convk_w"], f["convk_b"],
                                 f["kproj_w"], f["kproj_b"])

    common = {
        "EqT": EqT, "bq": bq, "EkT": EkT, "bk": bk,
        "WvT": f["Wv_w"].T, "Wv_b": f["Wv_b"],
        "tau1p": np.concatenate([f["tau1_w"][:, 0, :], f["tau1_b"][:, None]], axis=1),
        "del1p": np.concatenate([f["del1_w"][:, 0, :], f["del1_b"][:, None]], axis=1),
        "tau2T": f["tau2_w"][:, :, 0].T, "tau2_b": f["tau2_b"],
        "del2T": f["del2_w"][:, :, 0].T, "del2_b": f["del2_b"],
        "out_b": f["out_b"],
    }
    perm = np.concatenate([gr * 128 + np.concatenate([np.arange(0, 128, 2), np.arange(1, 128, 2)])
                           for gr in range(16)])
    for k in ["tau1p", "del1p", "tau2T", "del2T"]:
        common[k] = np.asarray(common[k])[perm]
    common = {k: np.ascontiguousarray(np.asarray(v, dtype=np.float32)) for k, v in common.items()}
    common["outTb"] = np.ascontiguousarray(f["out_w"].T).astype(ml_dtypes.bfloat16)
    for k in ["tau2T", "del2T"]:
        common[k] = common[k].astype(ml_dtypes.bfloat16)

    ins = []
    for c in range(NCORES):
        b, t0 = c // 4, (c % 4) * T
        xb = np.zeros((TH, D), np.float32)
        lo, hi = max(t0 - 1, 0), min(t0 + T + 1, L)
        xb[lo - (t0 - 1):hi - (t0 - 1)] = x[b, lo:hi]
        m = dict(common)
        m["xT"] = np.ascontiguousarray(xb.T)
        eq = np.zeros((D, 2), np.float32)
        ek = np.zeros((D, 2), np.float32)
        if t0 == 0:
            eq[:, 0], ek[:, 0] = eq_lo, ek_lo
        if t0 + T == L:
            eq[:, 1], ek[:, 1] = eq_hi, ek_hi
        m["eq_edge"] = eq
        m["ek_edge"] = ek
        ins.append(m)
    return ins


def assemble(results):
    y = np.empty((B, L, D), np.float32)
    for c in range(NCORES):
        yT = results[c]["yT"]
        y[0, c * 256:(c + 1) * 256] = yT[:, 0:256].T
        y[1, c * 256:(c + 1) * 256] = yT[:, 256:512].T
    return y


def kernel(**inputs):
    """Takes the full unsharded reference inputs, returns the full (B, L, D) output."""
    from concourse.bass_utils import run_bass_kernel_spmd
    nc, _ = build()
    in_maps = make_inputs(inputs)
    res = run_bass_kernel_spmd(nc, in_maps, list(range(NCORES)))
    return assemble(res.results)
